# revision 19
# baseline (speedup 1.0000x reference)
"""Trainium2 Bass kernel for Luong-attention (nn_Attention_4174708212176).

out[b] = softmax(dec[b] @ (enc[b] @ W)^T) @ enc[b],  b = 0..7, one batch per core.

v2 design (vs. the hi/lo split baseline): everything runs in single-pass
fp16 (validated on the real inputs: rel err 9.4e-3 < 2e-2 gate):
- M1 (enc @ W -> epT):    fp16, one accumulating pass, 32768 PE rows.
- M2 (dec @ ep^T):        fp16, one pass, 8192 rows/tile.
- M3 (P @ enc):           fp16, 8192 rows/tile.
- P^T via PE transposes:  fp16, 2048 rows/tile.
Total PE ~328k rows (~137 us @ 2.4 GHz) vs ~459k for the baseline.

Softmax is flash-style per 512-chunk so logits PSUM banks release
incrementally (4 banks would otherwise collide with the next tile's M2):
per chunk c: DVE local max m_c -> ACT exp(l - m_c) with fp16 out +
accumulated sums; then scale_c = exp(m_c - M) (ACT, 4-wide), p-chunk
rescale on DVE (4x fp16 mode), fused sum-of-sums via tensor_tensor_reduce,
reciprocal folded into the M3 PSUM->SBUF copy.

Engine balance per tile (PE 7.7 us): DVE ~4.6 us (maxes, rescales, 2 PT
copies), ACT ~3.7 us (exps, out copy), Pool ~1.4 us (2 PT copies).
Output is stored fp16 (halves out DMA; adds ~5e-4 rel err).
"""
import contextlib
import numpy as np

import concourse.bass as bass
import concourse.tile as tile
from concourse import bacc, mybir
from concourse.bass_utils import run_bass_kernel_spmd
from concourse.masks import make_identity

B, S, T, E, D = 8, 2048, 2048, 512, 512
P = 128
DO = D // P      # 4  d-tiles
EO = E // P      # 4  e-tiles
SO = S // P      # 16 s-tiles
TO = T // P      # 16 t-tiles
SC = S // 512    # 4  512-wide s-chunks
NCORES = 8

# packed free-dim offsets (fp16 elements per partition)
OFF_W = 0                    # W       [4, 512]
OFF_ET = OFF_W + EO * D      # encT    [4, 2048] sc-major
OFF_DT = OFF_ET + EO * S     # decT    [4, 2048]
OFF_EN = OFF_DT + DO * T     # enc natural [16, 512]
FREE = OFF_EN + SO * E

SEGS = [  # (name, offset, width)
    ("w", OFF_W, EO * D),
    ("et", OFF_ET, EO * S),
    ("dt", OFF_DT, DO * T),
    ("en", OFF_EN, SO * E),
]

_compiled_nc = {}


def _build(reps=1):
    nc = bacc.Bacc()
    x_in = nc.declare_dram_parameter("x", [P, FREE], mybir.dt.float16, isOutput=False)
    out_d = nc.declare_dram_parameter("out", [T, E], mybir.dt.float16, isOutput=True)

    with tile.TileContext(nc) as tc:
        with tc.tile_pool(name="const", bufs=1) as cpool, \
             tc.tile_pool(name="seg", bufs=2) as segpool, \
             tc.tile_pool(name="ep", bufs=1) as eppool, \
             tc.tile_pool(name="work", bufs=4) as wpool, \
             tc.tile_pool(name="stat", bufs=4) as spool, \
             tc.tile_pool(name="psA", bufs=6, space="PSUM") as psA, \
             tc.tile_pool(name="psC", bufs=1, space="PSUM") as psC:

            ident = cpool.tile([P, P], mybir.dt.float16)
            make_identity(nc, ident[:])

            _ENGS = (mybir.EngineType.PE, mybir.EngineType.Activation,
                     mybir.EngineType.DVE, mybir.EngineType.SP,
                     mybir.EngineType.Pool)
            loop_ctx = (tc.For_i(0, reps, 1, hint_engines=_ENGS)
                        if reps > 1 else contextlib.nullcontext())
            with loop_ctx:
                _body(nc, tc, segpool, eppool, wpool, spool,
                      psA, psC, x_in, out_d, ident)

    nc.compile()
    return nc


def _body(nc, tc, segpool, eppool, wpool, spool, psA, psC,
          x_in, out_d, ident):
    seg = {}
    for name, off, width in SEGS:
        seg[name] = segpool.tile([P, width], mybir.dt.float16, tag=f"seg_{name}",
                                 name=f"seg_{name}")
    segd = dict((n, (o, w)) for n, o, w in SEGS)
    nc.sync.dma_start(seg["w"][:], x_in.ap()[:, segd["w"][0]:segd["w"][0] + segd["w"][1]])
    # encT per-sc chunks so M1 starts as soon as chunk 0 lands
    for sc in range(SC):
        off, width = segd["et"]
        w4 = width // SC
        nc.sync.dma_start(seg["et"][:, sc * w4:(sc + 1) * w4],
                          x_in.ap()[:, off + sc * w4:off + (sc + 1) * w4])
    for name in ("dt", "en"):
        off, width = segd[name]
        nc.sync.dma_start(seg[name][:], x_in.ap()[:, off:off + width])

    def wh(eo, do):  # W tile [128, 128] (lhsT for M1)
        o = eo * D + do * P
        return seg["w"][:, o:o + P]

    def eth(eo, sc):  # encT chunk [128, 512] (rhs for M1), sc-major
        o = sc * 4 * 512 + eo * 512
        return seg["et"][:, o:o + 512]

    def dth(do, tt):  # decT tile [128, 128] (lhsT for M2)
        o = do * T + tt * P
        return seg["dt"][:, o:o + P]

    def encn(st):  # enc natural tile [128, 512] (rhs for M3)
        o = st * E
        return seg["en"][:, o:o + 512]

    # ---- M1: epT[d, s] = sum_e W[e, d] * encT[e, s], single fp16 pass
    eph = eppool.tile([P, DO * S], mybir.dt.float16)  # [128, 4*2048]
    i = 0
    for sc in range(SC):
        for do in range(DO):
            ps = psA.tile([P, 512], mybir.dt.float32, tag="ps_l",
                          name=f"m1_{do}_{sc}")
            for eo in range(EO):
                nc.tensor.matmul(ps[:], wh(eo, do), eth(eo, sc),
                                 start=(eo == 0), stop=(eo == EO - 1),
                                 skip_group_check=True)
            dst = slice(do * S + sc * 512, do * S + sc * 512 + 512)
            if i % 2 == 0:
                nc.scalar.copy(eph[:, dst], ps[:])
            else:
                nc.vector.tensor_copy(eph[:, dst], ps[:])
            i += 1

    def ephc(do, sc):
        o = do * S + sc * 512
        return eph[:, o:o + 512]

    # ---- per t-tile: M2 logits -> chunked softmax; transpose+M3 of the
    # previous tile are emitted after the next tile's M2 so the PE never
    # waits on the softmax engines.
    def emit_m2_softmax(tt):
        p_sb = wpool.tile([P, S], mybir.dt.float16, name=f"p{tt}", tag="p")
        negm = spool.tile([P, SC], mybir.dt.float32, name=f"negm{tt}", tag="negm")
        sums = spool.tile([P, SC], mybir.dt.float32, name=f"sums{tt}", tag="sums")
        for sc in range(SC):
            ps = psA.tile([P, 512], mybir.dt.float32, tag="ps_l",
                          name=f"m2_{tt}_{sc}")
            for do in range(DO):
                nc.tensor.matmul(ps[:], dth(do, tt), ephc(do, sc),
                                 start=(do == 0), stop=(do == DO - 1),
                                 skip_group_check=True)
            # local max (negated) then exp(l - m_c) with accumulated sum
            nc.vector.tensor_reduce(negm[:, sc:sc + 1], ps[:],
                                    axis=mybir.AxisListType.X,
                                    op=mybir.AluOpType.max, negate=True)
            nc.scalar.activation(p_sb[:, sc * 512:(sc + 1) * 512], ps[:],
                                 mybir.ActivationFunctionType.Exp,
                                 bias=negm[:, sc:sc + 1], scale=1.0,
                                 accum_out=sums[:, sc:sc + 1])
        # global max M = -min(negm); scale_c = exp(m_c - M)
        negM = spool.tile([P, 1], mybir.dt.float32, name=f"negM{tt}", tag="negM")
        nc.vector.tensor_reduce(negM[:], negm[:], axis=mybir.AxisListType.X,
                                op=mybir.AluOpType.min)
        scl = spool.tile([P, SC], mybir.dt.float32, name=f"scl{tt}", tag="scl")
        nc.scalar.activation(scl[:], negm[:], mybir.ActivationFunctionType.Exp,
                             bias=negM[:], scale=-1.0)
        # s = sum_c sums_c * scale_c ; recip = 1/s
        sprod = spool.tile([P, SC], mybir.dt.float32, name=f"sprod{tt}", tag="sprod")
        ssum = spool.tile([P, 1], mybir.dt.float32, name=f"ssum{tt}", tag="ssum")
        nc.vector.tensor_tensor(sprod[:], sums[:], scl[:], mybir.AluOpType.mult)
        nc.vector.tensor_reduce(ssum[:], sprod[:], axis=mybir.AxisListType.X,
                                op=mybir.AluOpType.add)
        recip = spool.tile([P, 1], mybir.dt.float32, name=f"recip{tt}", tag="recip")
        nc.vector.reciprocal(recip[:], ssum[:])
        # rescale p chunks by scale_c (4x fp16 DVE mode)
        for sc in range(SC):
            nc.vector.tensor_scalar_mul(p_sb[:, sc * 512:(sc + 1) * 512],
                                        p_sb[:, sc * 512:(sc + 1) * 512],
                                        scl[:, sc:sc + 1])
        # P^T via the DMA xbar: one instruction turns p [128t, 2048s] into
        # 16 transposed [128s, 128t] blocks (the M3 lhsT layout).
        pt_sb = wpool.tile([P, SO * P], mybir.dt.float16, name=f"pt{tt}", tag="pt")
        nc.sync.dma_start_transpose(
            pt_sb[:].rearrange("Do (Di M) -> Do Di M", Di=SO), p_sb[:])
        return pt_sb, recip

    def emit_m3(tt, pt_sb, recip):
        # M3: out[t, e] = sum_s PT[s, t]^T * enc_n[s, e].
        # Two 8-long accumulation groups into two banks beat one 16-long
        # group (shorter groups measured faster); merge: ACT scales c1 by
        # 1/sum into SBUF, DVE adds (c0/sum) via scalar_tensor_tensor
        # (only one PSUM operand allowed per DVE op).
        c0 = psC.tile([P, E], mybir.dt.float32, tag="ps_out0", name=f"m3_{tt}_0")
        c1 = psC.tile([P, E], mybir.dt.float32, tag="ps_out1", name=f"m3_{tt}_1")
        for q, cq in enumerate((c0, c1)):
            for j in range(8):
                st = q * 8 + j
                nc.tensor.matmul(cq[:], pt_sb[:, st * P:(st + 1) * P],
                                 encn(st), start=(j == 0), stop=(j == 7))
        t_sb = wpool.tile([P, E], mybir.dt.float32, name=f"t{tt}", tag="t")
        nc.scalar.activation(t_sb[:], c1[:],
                             mybir.ActivationFunctionType.Copy,
                             bias=0.0, scale=recip[:])
        out_sb = wpool.tile([P, E], mybir.dt.float16, name=f"o{tt}", tag="o")
        nc.vector.scalar_tensor_tensor(out_sb[:], c0[:], recip[:], t_sb[:],
                                       mybir.AluOpType.mult,
                                       mybir.AluOpType.add)
        nc.sync.dma_start(out_d.ap()[tt * P:(tt + 1) * P, :], out_sb[:])

    # depth-2 pipeline: M3(t) runs during M2(t+2), giving the transpose DMA
    # a full tile-slot to complete without stalling the PE.
    hist = []
    for tt in range(TO):
        hist.append((tt, *emit_m2_softmax(tt)))
        if len(hist) > 2:
            emit_m3(*hist.pop(0))
    for item in hist:
        emit_m3(*item)


def _part(x, ko):
    """[K, F] -> [128, ko, F] -> [128, ko*F] flat, partition = k % 128."""
    kf = x.reshape(ko, P, -1).transpose(1, 0, 2)
    return np.ascontiguousarray(kf.reshape(P, -1))


def _f16(x):
    return x.astype(np.float16)


def _pack_core(enc_b, dec_b, wseg):
    encT = np.ascontiguousarray(enc_b.T)          # [512, 2048]
    eth = _f16(encT)
    dth = _f16(np.ascontiguousarray(dec_b.T))     # [512, 2048]
    en = _f16(enc_b)                              # [2048, 512]
    def scmajor(x):  # [128, EO*S] with [eo][sc][512] -> [sc][eo][512]
        v = _part(x, EO).reshape(P, EO, SC, 512)
        return np.ascontiguousarray(v.transpose(0, 2, 1, 3)).reshape(P, -1)

    segs = [wseg, scmajor(eth), _part(dth, DO), _part(en, SO)]
    return np.concatenate(segs, axis=1)


def _make_wseg(W):
    return _part(_f16(W), EO)


def make_in_maps(enc, dec, W):
    wseg = _make_wseg(W)
    return [{"x": _pack_core(enc[b], dec[b], wseg)} for b in range(NCORES)]


def kernel(enc_hidden_states, dec_hidden_states, W_att):
    enc = np.asarray(enc_hidden_states, np.float32)
    dec = np.asarray(dec_hidden_states, np.float32)
    W = np.asarray(W_att, np.float32)

    in_maps = make_in_maps(enc, dec, W)

    if 1 not in _compiled_nc:
        _compiled_nc[1] = _build(1)

    res = run_bass_kernel_spmd(_compiled_nc[1], in_maps, list(range(NCORES)))
    out = np.stack([res.results[b]["out"] for b in range(NCORES)], axis=0)
    return out.astype(np.float32)


if __name__ == "__main__":
    rng = np.random.default_rng(0)
    enc = rng.standard_normal((B, S, E), dtype=np.float32)
    dec = rng.standard_normal((B, T, D), dtype=np.float32)
    W = rng.standard_normal((E, D), dtype=np.float32)
    out = kernel(enc, dec, W)
    print("out", out.shape, out.dtype)
